# revision 2
# baseline (speedup 1.0000x reference)
"""Paged-attention decode (vLLM-style) Bass kernel for Trainium2, 8 NeuronCores.

Sharding: KV heads across the 8 cores (tensor-parallel). Core h owns kv head h
and query heads 4h..4h+3 for ALL 32 sequences, so every core runs an IDENTICAL
instruction stream (SPMD) — only its cache slice / q slice differ.

All K/V data is bf16 (tolerance is 2e-2 rel; bf16 rounding contributes ~5e-3).

Per core, host-side prep:
  - scatter the new k/v token into the caches (numpy), slice head h, cast bf16
  - K packed per block as [16 tok x 128 d] (4 KiB rows)
  - V packed per block as [16 tok x (128 d | 1 one | 7 pad)] (4.25 KiB rows);
    the ones column accumulates the softmax denominator during the PV matmul
  - per-sequence block lists -> int16 idx table (wrapped in 16 partitions,
    replicated for the 8 Q7 cores), a 0/1 token-validity mask table, and q^T

Device, per sequence, per 128-block gather (static schedule; counts baked in):
  - dma_gather(transpose=True) pulls K already TRANSPOSED:
    tile [128 d, 16 tok, 128 blk] -> slice [:, t, :] is K^T for token-offset t
  - dma_gather(transpose=False) pulls V: tile [128 blk, 2176]
  - per quad of 4 token-offsets: 4 matmuls st[128 blk, 16] = K^T q, one ACT
    exp -> bf16, one DVE mask-multiply, 4 PV matmuls o[4, 129] += w^T V
  - per sequence: copy o out of PSUM, reciprocal of the denominator column,
    multiply, DMA out.
"""

import numpy as np

B, H, HKV, D = 32, 32, 8, 128
NUM_BLOCKS, BLOCK_SIZE, MAX_NUM_BLOCKS = 4096, 16, 256
SCALE = 0.08838834764831845
NCORES = 8
G = H // HKV  # 4 query heads per kv head
BPG = 128  # blocks per gather
KROW = BLOCK_SIZE * D  # 2048 bf16 elems per K row
VTOK = D + 8  # 136: V(128) | ones-marker | 7 pad
VROWP = BLOCK_SIZE * VTOK  # 2176 bf16 elems per packed v row

LAST_EXEC_TIME_NS = None


def _plan(context_lens):
    nblocks = [int(-(-int(c) // BLOCK_SIZE)) if int(c) > 0 else 0 for c in context_lens]
    jobs = [b for b in range(B) if nblocks[b] > 0]
    ngathers = {b: -(-nblocks[b] // BPG) for b in jobs}
    return nblocks, jobs, ngathers


def _wrap16(ids):
    """[128] int16 -> [128, 8] wrapped in 16 partitions, replicated 8x."""
    wrapped = np.zeros((16, BPG // 16), np.int16)
    for i in range(BPG):
        wrapped[i % 16, i // 16] = ids[i]
    return np.tile(wrapped, (8, 1))


def _host_tables(block_tables, context_lens, nblocks, jobs, ngathers):
    """Block-idx table (block-0 pads up to n16), per-gather (cnt, n16) counts,
    expanded 0/1 token mask."""
    ng_total = sum(ngathers[b] for b in jobs)
    idx = np.full((128, ng_total * (BPG // 16)), -1, dtype=np.int16)
    counts = []
    mask = np.zeros((128, ng_total * BLOCK_SIZE * G), dtype=np.float32)
    col = 0
    p = np.arange(128)
    for b in jobs:
        nb = nblocks[b]
        ctx = int(context_lens[b])
        for g in range(ngathers[b]):
            lo = g * BPG
            n = min(BPG, nb - lo)
            n16 = -(-n // 16) * 16
            counts.append((n, n16))
            ids = np.full(BPG, -1, np.int16)
            ids[:n16] = 0
            ids[:n] = block_tables[b, lo : lo + n].astype(np.int16)
            cbase = col * (BPG // 16)
            idx[:, cbase : cbase + BPG // 16] = _wrap16(ids)
            # mask column layout: ((col*16 + t) * G + g') ; same value per g'
            for t in range(BLOCK_SIZE):
                valid = ((lo + p) * BLOCK_SIZE + t < ctx).astype(np.float32)
                mbase = (col * BLOCK_SIZE + t) * G
                for gg in range(G):
                    mask[:, mbase + gg] = valid
            col += 1
    return idx, counts, mask, ng_total


def _build_program(nblocks, jobs, ngathers, ng_total, counts, reps=1, mode="full"):
    import concourse.mybir as mybir
    import concourse.tile as tile
    from concourse import bacc

    do_dma = mode in ("full", "dma")
    do_compute = mode in ("full", "compute")

    f32 = mybir.dt.float32
    bf16 = mybir.dt.bfloat16
    i16 = mybir.dt.int16
    Exp = mybir.ActivationFunctionType.Exp
    mult = mybir.AluOpType.mult

    nj = len(jobs)
    nc = bacc.Bacc("TRN2", target_bir_lowering=False)

    with tile.TileContext(nc) as tc:
        with tc.tile_pool(name="dram", bufs=1, space="DRAM") as dram:
            kcache_t = dram.tile([NUM_BLOCKS, KROW], bf16,
                                 kind="ExternalInput", name="kcache", uniquify=False)
            vcache_t = dram.tile([NUM_BLOCKS, VROWP], bf16,
                                 kind="ExternalInput", name="vcache", uniquify=False)
            idx_t = dram.tile([128, ng_total * (BPG // 16)], i16,
                              kind="ExternalInput", name="idx", uniquify=False)
            mask_t = dram.tile([128, ng_total * BLOCK_SIZE * G], bf16,
                               kind="ExternalInput", name="mask", uniquify=False)
            qq_t = dram.tile([D, B * G], bf16, kind="ExternalInput", name="qq", uniquify=False)
            o_t = dram.tile([nj, G, D], f32, kind="ExternalOutput", name="o", uniquify=False)

        with (
            tc.tile_pool(name="resident", bufs=1) as rpool,
            tc.tile_pool(name="kpool", bufs=4) as kpool,
            tc.tile_pool(name="vpool", bufs=4) as vpool,
            tc.tile_pool(name="wpool", bufs=8) as wpool,
            tc.tile_pool(name="small", bufs=2) as small_pool,
            tc.tile_pool(name="stps", bufs=4, space="PSUM") as stps_pool,
            tc.tile_pool(name="ops", bufs=2, space="PSUM") as ops_pool,
        ):
            idx_sb = rpool.tile([128, ng_total * (BPG // 16)], i16, tag="idx", name="idx_sb")
            mask_sb = rpool.tile([128, ng_total * BLOCK_SIZE * G], bf16, tag="mask", name="mask_sb")
            qq_sb = rpool.tile([D, B * G], bf16, tag="qq", name="qq_sb")
            nc.sync.dma_start(idx_sb[:], idx_t[:])
            nc.sync.dma_start(mask_sb[:], mask_t[:])
            nc.sync.dma_start(qq_sb[:], qq_t[:])

            for _rep in range(reps):
                col = 0
                gi = 0
                for jb, b in enumerate(jobs):
                    o_ps = ops_pool.tile([G, D + 1], f32, tag="o")
                    nq_total = ngathers[b] * 4  # quads per sequence
                    qi = 0
                    for g in range(ngathers[b]):
                        cnt, n = counts[gi]
                        ktile = kpool.tile([128, BLOCK_SIZE, BPG], bf16, tag="k")
                        vtile = vpool.tile([128, 1, VROWP], bf16, tag="v")
                        if do_dma:
                            nc.gpsimd.dma_gather(
                                ktile[:], kcache_t[:],
                                idx_sb[:, col * 8 : (col + 1) * 8],
                                BPG, n, KROW, transpose=True,
                            )
                            nc.gpsimd.dma_gather(
                                vtile[:], vcache_t[:],
                                idx_sb[:, col * 8 : (col + 1) * 8],
                                BPG, n, VROWP,
                            )
                        if not do_compute:
                            col += 1
                            gi += 1
                            continue
                        for q4 in range(4):
                            first = qi == 0
                            last = qi == nq_total - 1
                            st4 = stps_pool.tile([128, 4 * G], f32, tag="st")
                            for u in range(4):
                                t = q4 * 4 + u
                                nc.tensor.matmul(
                                    st4[:n, u * G : (u + 1) * G],
                                    lhsT=ktile[:, t, :n],
                                    rhs=qq_sb[:, b * G : (b + 1) * G],
                                    start=True, stop=True,
                                )
                            e4 = wpool.tile([128, 4 * G], f32, tag="e4")
                            nc.scalar.activation(e4[:n], st4[:n], Exp, scale=SCALE)
                            wt4 = wpool.tile([128, 4 * G], bf16, tag="wt")
                            mbase = (col * BLOCK_SIZE + q4 * 4) * G
                            nc.vector.tensor_tensor(
                                out=wt4[:n], in0=e4[:n],
                                in1=mask_sb[:n, mbase : mbase + 4 * G],
                                op=mult,
                            )
                            for u in range(4):
                                t = q4 * 4 + u
                                w = wt4[:n, u * G : (u + 1) * G]
                                vh = vtile[:n, 0, t * VTOK : t * VTOK + D + 1]
                                nc.tensor.matmul(
                                    o_ps[:], lhsT=w, rhs=vh,
                                    start=first and u == 0,
                                    stop=last and u == 3,
                                )
                            qi += 1
                        col += 1
                        gi += 1
                    if not do_compute:
                        continue
                    # per-sequence epilogue: divide by denominator, store
                    o_sb = small_pool.tile([G, D + 1], f32, tag="osb")
                    nc.vector.tensor_copy(o_sb[:], o_ps[:])
                    rec_sb = small_pool.tile([G, 1], f32, tag="rec")
                    nc.vector.reciprocal(rec_sb[:], o_sb[:, D : D + 1])
                    oo_sb = small_pool.tile([G, D], f32, tag="oosb")
                    nc.vector.tensor_scalar(
                        oo_sb[:], o_sb[:, 0:D], rec_sb[:], None, op0=mult
                    )
                    nc.sync.dma_start(o_t[jb], oo_sb[:])

    nc.compile()
    return nc


def _host_prep(q, k, v, k_cache, v_cache, slot_mapping):
    """Returns per-core caches and q slices (all bf16)."""
    import ml_dtypes

    bf16 = ml_dtypes.bfloat16
    kc = k_cache.reshape(-1, HKV, D).copy()
    vc = v_cache.reshape(-1, HKV, D).copy()
    kc[slot_mapping] = k
    vc[slot_mapping] = v
    kc = kc.reshape(NUM_BLOCKS, BLOCK_SIZE, HKV, D)
    vc = vc.reshape(NUM_BLOCKS, BLOCK_SIZE, HKV, D)
    per_core = []
    for h in range(NCORES):
        kcache_h = np.ascontiguousarray(
            kc[:, :, h, :].reshape(NUM_BLOCKS, KROW)
        ).astype(bf16)
        vh = vc[:, :, h, :].astype(bf16)
        vcache_h = np.zeros((NUM_BLOCKS, BLOCK_SIZE, VTOK), dtype=bf16)
        vcache_h[:, :, :D] = vh
        vcache_h[:, :, D] = 1.0
        vcache_h = vcache_h.reshape(NUM_BLOCKS, VROWP)
        qT_h = np.ascontiguousarray(
            q[:, h * G : (h + 1) * G, :].transpose(2, 0, 1).reshape(D, B * G)
        ).astype(bf16)
        per_core.append((kcache_h, vcache_h, qT_h))
    return per_core


def make_in_maps(q, k, v, k_cache, v_cache, slot_mapping, idx, mask):
    import ml_dtypes

    per_core = _host_prep(q, k, v, k_cache, v_cache, slot_mapping)
    mask_bf = mask.astype(ml_dtypes.bfloat16)
    in_maps = []
    for h in range(NCORES):
        kcache_h, vcache_h, qq = per_core[h]
        in_maps.append(
            {
                "kcache": kcache_h,
                "vcache": vcache_h,
                "idx": idx,
                "mask": mask_bf,
                "qq": qq,
            }
        )
    return in_maps


def assemble(results, jobs, context_lens):
    out = np.zeros((B, 1, H, D), dtype=np.float32)
    for h in range(NCORES):
        o_h = results[h]["o"]  # [nj, G, D]
        for jb, b in enumerate(jobs):
            if int(context_lens[b]) <= 0:
                continue
            out[b, 0, h * G : (h + 1) * G, :] = o_h[jb]
    return out


def kernel(q, k, v, k_cache, v_cache, slot_mapping, block_tables, context_lens):
    global LAST_EXEC_TIME_NS
    q = np.asarray(q, dtype=np.float32)
    k = np.asarray(k, dtype=np.float32)
    v = np.asarray(v, dtype=np.float32)
    k_cache = np.asarray(k_cache, dtype=np.float32)
    v_cache = np.asarray(v_cache, dtype=np.float32)
    slot_mapping = np.asarray(slot_mapping, dtype=np.int32)
    block_tables = np.asarray(block_tables, dtype=np.int32)
    context_lens = np.asarray(context_lens, dtype=np.int32)

    nblocks, jobs, ngathers = _plan(context_lens)
    if not jobs:
        return np.zeros((B, 1, H, D), dtype=np.float32)

    idx, counts, mask, ng_total = _host_tables(
        block_tables, context_lens, nblocks, jobs, ngathers
    )
    in_maps = make_in_maps(q, k, v, k_cache, v_cache, slot_mapping, idx, mask)
    nc = _build_program(nblocks, jobs, ngathers, ng_total, counts)

    from concourse.bass_utils import run_bass_kernel_spmd

    res = run_bass_kernel_spmd(nc, in_maps, core_ids=list(range(NCORES)))
    LAST_EXEC_TIME_NS = res.exec_time_ns
    return assemble(res.results, jobs, context_lens)


# revision 3
# speedup vs baseline: 1.0087x; 1.0087x over previous
"""Paged-attention decode (vLLM-style) Bass kernel for Trainium2, 8 NeuronCores.

Sharding: KV heads across the 8 cores (tensor-parallel). Core h owns kv head h
and query heads 4h..4h+3 for ALL 32 sequences, so every core runs an IDENTICAL
instruction stream (SPMD) — only its cache slice / q slice differ.

All K/V data is bf16 (tolerance is 2e-2 rel; bf16 rounding contributes ~3e-3).

Per core, host-side prep:
  - scatter the new k/v token into the caches (numpy), slice head h, cast bf16
  - K packed per block as [16 tok x 128 d] (4 KiB rows)
  - V packed per block as [16 tok x (128 d | 1 one | 7 pad)] (4.25 KiB rows);
    the ones column accumulates the softmax denominator during the PV matmul
  - per-sequence block lists -> int16 idx table (wrapped in 16 partitions,
    replicated for the 8 Q7 cores), a 0/1 token-validity mask table, and q^T

Device, one gather pair per sequence (up to 256 blocks each; static schedule):
  - dma_gather(transpose=True) pulls K already TRANSPOSED:
    tile [128 d, 16 tok, 256 blk] -> slice [:, t, :] is K^T for token-offset t
  - dma_gather(transpose=False) pulls V: tile [128 blk, 2, 2176]
  - per 128-block group, per quad of 4 token-offsets: 4 matmuls
    st[128 blk, 16] = K^T q, one ACT exp -> bf16, one DVE mask-multiply,
    4 PV matmuls o[4, 129] += w^T V
  - per sequence: copy o out of PSUM, reciprocal of the denominator column,
    multiply, DMA out.
"""

import numpy as np

B, H, HKV, D = 32, 32, 8, 128
NUM_BLOCKS, BLOCK_SIZE, MAX_NUM_BLOCKS = 4096, 16, 256
SCALE = 0.08838834764831845
NCORES = 8
G = H // HKV  # 4 query heads per kv head
BPG = 256  # blocks per gather (= max blocks per sequence)
KROW = BLOCK_SIZE * D  # 2048 bf16 elems per K row
VTOK = D + 8  # 136: V(128) | ones-marker | 7 pad
VROWP = BLOCK_SIZE * VTOK  # 2176 bf16 elems per packed v row

LAST_EXEC_TIME_NS = None


def _plan(context_lens):
    nblocks = [int(-(-int(c) // BLOCK_SIZE)) if int(c) > 0 else 0 for c in context_lens]
    jobs = [b for b in range(B) if nblocks[b] > 0]
    ngathers = {b: 1 for b in jobs}  # one (K, V) gather pair per sequence
    return nblocks, jobs, ngathers


def _wrap16(ids):
    """[BPG] int16 -> [128, BPG//16] wrapped in 16 partitions, replicated 8x."""
    wrapped = np.zeros((16, BPG // 16), np.int16)
    for i in range(BPG):
        wrapped[i % 16, i // 16] = ids[i]
    return np.tile(wrapped, (8, 1))


def _host_tables(block_tables, context_lens, nblocks, jobs, ngathers):
    """Block-idx table (block-0 pads up to n16), per-job (cnt, n16) counts,
    expanded 0/1 token mask per 128-block group."""
    nj = len(jobs)
    ng128 = sum(-(-min(nblocks[b] // 16 * 16 + 16 * (nblocks[b] % 16 > 0), BPG) // 128)
                for b in jobs)  # number of 128-block groups (recomputed below)
    # build counts first
    counts = []
    groups_per_job = []
    for b in jobs:
        nb = nblocks[b]
        n16 = -(-nb // 16) * 16
        counts.append((nb, n16))
        groups_per_job.append(-(-n16 // 128))
    ng128 = sum(groups_per_job)

    idx = np.full((128, nj * (BPG // 16)), 0, dtype=np.int16)
    mask = np.zeros((128, ng128 * BLOCK_SIZE * G), dtype=np.float32)
    p = np.arange(128)
    col128 = 0
    for j, b in enumerate(jobs):
        nb = nblocks[b]
        ctx = int(context_lens[b])
        ids = np.zeros(BPG, np.int16)
        ids[:nb] = block_tables[b, :nb].astype(np.int16)
        idx[:, j * (BPG // 16) : (j + 1) * (BPG // 16)] = _wrap16(ids)
        for g in range(groups_per_job[j]):
            lo = g * 128
            for t in range(BLOCK_SIZE):
                valid = ((lo + p) * BLOCK_SIZE + t < ctx).astype(np.float32)
                mbase = (col128 * BLOCK_SIZE + t) * G
                for gg in range(G):
                    mask[:, mbase + gg] = valid
            col128 += 1
    return idx, counts, mask, ng128


def _build_program(nblocks, jobs, ngathers, ng128, counts, reps=1, mode="full"):
    import concourse.mybir as mybir
    import concourse.tile as tile
    from concourse import bacc

    do_dma = mode in ("full", "dma")
    do_compute = mode in ("full", "compute")

    f32 = mybir.dt.float32
    bf16 = mybir.dt.bfloat16
    i16 = mybir.dt.int16
    Exp = mybir.ActivationFunctionType.Exp
    mult = mybir.AluOpType.mult

    nj = len(jobs)
    nc = bacc.Bacc("TRN2", target_bir_lowering=False)

    with tile.TileContext(nc) as tc:
        with tc.tile_pool(name="dram", bufs=1, space="DRAM") as dram:
            kcache_t = dram.tile([NUM_BLOCKS, KROW], bf16,
                                 kind="ExternalInput", name="kcache", uniquify=False)
            vcache_t = dram.tile([NUM_BLOCKS, VROWP], bf16,
                                 kind="ExternalInput", name="vcache", uniquify=False)
            idx_t = dram.tile([128, nj * (BPG // 16)], i16,
                              kind="ExternalInput", name="idx", uniquify=False)
            mask_t = dram.tile([128, ng128 * BLOCK_SIZE * G], bf16,
                               kind="ExternalInput", name="mask", uniquify=False)
            qq_t = dram.tile([D, B * G], bf16, kind="ExternalInput", name="qq", uniquify=False)
            o_t = dram.tile([nj, G, D], f32, kind="ExternalOutput", name="o", uniquify=False)

        with (
            tc.tile_pool(name="resident", bufs=1) as rpool,
            tc.tile_pool(name="kpool", bufs=3) as kpool,
            tc.tile_pool(name="vpool", bufs=3) as vpool,
            tc.tile_pool(name="wpool", bufs=8) as wpool,
            tc.tile_pool(name="small", bufs=2) as small_pool,
            tc.tile_pool(name="stps", bufs=4, space="PSUM") as stps_pool,
            tc.tile_pool(name="ops", bufs=2, space="PSUM") as ops_pool,
        ):
            idx_sb = rpool.tile([128, nj * (BPG // 16)], i16, tag="idx", name="idx_sb")
            mask_sb = rpool.tile([128, ng128 * BLOCK_SIZE * G], bf16, tag="mask", name="mask_sb")
            qq_sb = rpool.tile([D, B * G], bf16, tag="qq", name="qq_sb")
            nc.sync.dma_start(idx_sb[:], idx_t[:])
            nc.sync.dma_start(mask_sb[:], mask_t[:])
            nc.sync.dma_start(qq_sb[:], qq_t[:])

            for _rep in range(reps):
                col128 = 0
                for jb, b in enumerate(jobs):
                    cnt, n16 = counts[jb]
                    ngroups = -(-n16 // 128)
                    o_ps = ops_pool.tile([G, D + 1], f32, tag="o")
                    ktile = kpool.tile([128, BLOCK_SIZE, BPG], bf16, tag="k")
                    vtile = vpool.tile([128, BPG // 128, VROWP], bf16, tag="v")
                    if do_dma:
                        nc.gpsimd.dma_gather(
                            ktile[:], kcache_t[:],
                            idx_sb[:, jb * 16 : (jb + 1) * 16],
                            BPG, n16, KROW, transpose=True,
                        )
                        nc.gpsimd.dma_gather(
                            vtile[:], vcache_t[:],
                            idx_sb[:, jb * 16 : (jb + 1) * 16],
                            BPG, n16, VROWP,
                        )
                    if not do_compute:
                        col128 += ngroups
                        continue
                    nq_total = ngroups * 4
                    qi = 0
                    for grp in range(ngroups):
                        nh = min(128, n16 - grp * 128)
                        for q4 in range(4):
                            first = qi == 0
                            last = qi == nq_total - 1
                            st4 = stps_pool.tile([128, 4 * G], f32, tag="st")
                            for u in range(4):
                                t = q4 * 4 + u
                                nc.tensor.matmul(
                                    st4[:nh, u * G : (u + 1) * G],
                                    lhsT=ktile[:, t, grp * 128 : grp * 128 + nh],
                                    rhs=qq_sb[:, b * G : (b + 1) * G],
                                    start=True, stop=True,
                                )
                            e4 = wpool.tile([128, 4 * G], f32, tag="e4")
                            nc.scalar.activation(e4[:nh], st4[:nh], Exp, scale=SCALE)
                            wt4 = wpool.tile([128, 4 * G], bf16, tag="wt")
                            mbase = (col128 * BLOCK_SIZE + q4 * 4) * G
                            nc.vector.tensor_tensor(
                                out=wt4[:nh], in0=e4[:nh],
                                in1=mask_sb[:nh, mbase : mbase + 4 * G],
                                op=mult,
                            )
                            for u in range(4):
                                t = q4 * 4 + u
                                w = wt4[:nh, u * G : (u + 1) * G]
                                vh = vtile[:nh, grp, t * VTOK : t * VTOK + D + 1]
                                nc.tensor.matmul(
                                    o_ps[:], lhsT=w, rhs=vh,
                                    start=first and u == 0,
                                    stop=last and u == 3,
                                )
                            qi += 1
                        col128 += 1
                    # per-sequence epilogue: divide by denominator, store
                    o_sb = small_pool.tile([G, D + 1], f32, tag="osb")
                    nc.vector.tensor_copy(o_sb[:], o_ps[:])
                    rec_sb = small_pool.tile([G, 1], f32, tag="rec")
                    nc.vector.reciprocal(rec_sb[:], o_sb[:, D : D + 1])
                    oo_sb = small_pool.tile([G, D], f32, tag="oosb")
                    nc.vector.tensor_scalar(
                        oo_sb[:], o_sb[:, 0:D], rec_sb[:], None, op0=mult
                    )
                    nc.sync.dma_start(o_t[jb], oo_sb[:])

    nc.compile()
    return nc


def _host_prep(q, k, v, k_cache, v_cache, slot_mapping):
    """Returns per-core caches and q slices (all bf16)."""
    import ml_dtypes

    bf16 = ml_dtypes.bfloat16
    kc = k_cache.reshape(-1, HKV, D).copy()
    vc = v_cache.reshape(-1, HKV, D).copy()
    kc[slot_mapping] = k
    vc[slot_mapping] = v
    kc = kc.reshape(NUM_BLOCKS, BLOCK_SIZE, HKV, D)
    vc = vc.reshape(NUM_BLOCKS, BLOCK_SIZE, HKV, D)
    per_core = []
    for h in range(NCORES):
        kcache_h = np.ascontiguousarray(
            kc[:, :, h, :].reshape(NUM_BLOCKS, KROW)
        ).astype(bf16)
        vh = vc[:, :, h, :].astype(bf16)
        vcache_h = np.zeros((NUM_BLOCKS, BLOCK_SIZE, VTOK), dtype=bf16)
        vcache_h[:, :, :D] = vh
        vcache_h[:, :, D] = 1.0
        vcache_h = vcache_h.reshape(NUM_BLOCKS, VROWP)
        qT_h = np.ascontiguousarray(
            q[:, h * G : (h + 1) * G, :].transpose(2, 0, 1).reshape(D, B * G)
        ).astype(bf16)
        per_core.append((kcache_h, vcache_h, qT_h))
    return per_core


def make_in_maps(q, k, v, k_cache, v_cache, slot_mapping, idx, mask):
    import ml_dtypes

    per_core = _host_prep(q, k, v, k_cache, v_cache, slot_mapping)
    mask_bf = mask.astype(ml_dtypes.bfloat16)
    in_maps = []
    for h in range(NCORES):
        kcache_h, vcache_h, qq = per_core[h]
        in_maps.append(
            {
                "kcache": kcache_h,
                "vcache": vcache_h,
                "idx": idx,
                "mask": mask_bf,
                "qq": qq,
            }
        )
    return in_maps


def assemble(results, jobs, context_lens):
    out = np.zeros((B, 1, H, D), dtype=np.float32)
    for h in range(NCORES):
        o_h = results[h]["o"]  # [nj, G, D]
        for jb, b in enumerate(jobs):
            if int(context_lens[b]) <= 0:
                continue
            out[b, 0, h * G : (h + 1) * G, :] = o_h[jb]
    return out


def kernel(q, k, v, k_cache, v_cache, slot_mapping, block_tables, context_lens):
    global LAST_EXEC_TIME_NS
    q = np.asarray(q, dtype=np.float32)
    k = np.asarray(k, dtype=np.float32)
    v = np.asarray(v, dtype=np.float32)
    k_cache = np.asarray(k_cache, dtype=np.float32)
    v_cache = np.asarray(v_cache, dtype=np.float32)
    slot_mapping = np.asarray(slot_mapping, dtype=np.int32)
    block_tables = np.asarray(block_tables, dtype=np.int32)
    context_lens = np.asarray(context_lens, dtype=np.int32)

    nblocks, jobs, ngathers = _plan(context_lens)
    if not jobs:
        return np.zeros((B, 1, H, D), dtype=np.float32)

    idx, counts, mask, ng128 = _host_tables(
        block_tables, context_lens, nblocks, jobs, ngathers
    )
    in_maps = make_in_maps(q, k, v, k_cache, v_cache, slot_mapping, idx, mask)
    nc = _build_program(nblocks, jobs, ngathers, ng128, counts)

    from concourse.bass_utils import run_bass_kernel_spmd

    res = run_bass_kernel_spmd(nc, in_maps, core_ids=list(range(NCORES)))
    LAST_EXEC_TIME_NS = res.exec_time_ns
    return assemble(res.results, jobs, context_lens)


# revision 13
# speedup vs baseline: 7.5976x; 7.5321x over previous
"""Paged-attention decode (vLLM-style) Bass kernel for Trainium2, 8 NeuronCores.

Sharding: KV heads across the 8 cores (tensor-parallel). Core h owns kv head h
and query heads 4h..4h+3 for ALL 32 sequences, so every core runs an IDENTICAL
instruction stream (SPMD) — only its cache slice / q slice differ.

All K/V data is bf16 (tolerance is 2e-2 rel; bf16 rounding contributes ~3e-3).

Per core, host-side prep:
  - scatter the new k/v token into the caches (numpy), slice head h, cast bf16
  - K packed per block as [16 tok x 128 d] (4 KiB rows)
  - V packed per block as [16 tok x 128 d] (4 KiB rows)
  - per-sequence block lists -> int16 idx table (wrapped in 16 partitions,
    replicated for the 8 Q7 cores), a 0/1 token-validity mask table, q^T, and
    a [16,4] fold matrix that sums the 16 (offset, head) denominator rows per
    head

Device, one gather pair per sequence (up to 256 blocks each; static schedule):
  - dma_gather(transpose=True) pulls K already TRANSPOSED:
    tile [128 d, 16 tok, 256 blk] -> slice [:, t, :] is K^T for token-offset t
  - dma_gather(transpose=False) pulls V: tile [128 blk, 2, 2048]
  - per 128-block group: 16 matmuls st[128 blk, 64] = K^T q (one per token
    offset), one ACT exp over [128, 64], one DVE mask-multiply -> bf16 w,
    16 PV matmuls o[4, 128] += w^T V, 1 denominator matmul den[64,1] += w^T 1
  - per sequence: copy o/den out of PSUM, fold den 64->4 (matmul),
    reciprocal, multiply, DMA out.
"""

import numpy as np

B, H, HKV, D = 32, 32, 8, 128
NUM_BLOCKS, BLOCK_SIZE, MAX_NUM_BLOCKS = 4096, 16, 256
SCALE = 0.08838834764831845
NCORES = 8
G = H // HKV  # 4 query heads per kv head
BPG = 256  # blocks per gather (= max blocks per sequence)
KROW = BLOCK_SIZE * D  # 2048 bf16 elems per K row
VROW = BLOCK_SIZE * D  # 2048 bf16 elems per V row

LAST_EXEC_TIME_NS = None


def _plan(context_lens):
    nblocks = [int(-(-int(c) // BLOCK_SIZE)) if int(c) > 0 else 0 for c in context_lens]
    jobs = [b for b in range(B) if nblocks[b] > 0]
    ngathers = {b: 1 for b in jobs}  # one (K, V) gather pair per sequence
    return nblocks, jobs, ngathers


def _wrap16(ids):
    """[BPG] int16 -> [128, BPG//16] wrapped in 16 partitions, replicated 8x."""
    wrapped = np.zeros((16, BPG // 16), np.int16)
    for i in range(BPG):
        wrapped[i % 16, i // 16] = ids[i]
    return np.tile(wrapped, (8, 1))


def _host_tables(block_tables, context_lens, nblocks, jobs, ngathers):
    """Block-idx table, per-job (cnt, n16) counts, expanded 0/1 token mask per
    128-block group."""
    nj = len(jobs)
    counts = []
    groups_per_job = []
    for b in jobs:
        nb = nblocks[b]
        n16 = -(-nb // 16) * 16
        counts.append((nb, n16))
        groups_per_job.append(-(-n16 // 128))
    ng128 = sum(groups_per_job)

    idx = np.zeros((128, nj * (BPG // 16)), dtype=np.int16)
    mask = np.zeros((128, ng128 * BLOCK_SIZE * G), dtype=np.float32)
    p = np.arange(128)
    col128 = 0
    for j, b in enumerate(jobs):
        nb = nblocks[b]
        ctx = int(context_lens[b])
        n16 = -(-nb // 16) * 16
        ids = np.full(BPG, -1, np.int16)
        ids[:n16] = 0
        ids[:nb] = block_tables[b, :nb].astype(np.int16)
        idx[:, j * (BPG // 16) : (j + 1) * (BPG // 16)] = _wrap16(ids)
        for g in range(groups_per_job[j]):
            lo = g * 128
            for t in range(BLOCK_SIZE):
                valid = ((lo + p) * BLOCK_SIZE + t < ctx).astype(np.float32)
                mbase = (col128 * BLOCK_SIZE + t) * G
                for gg in range(G):
                    mask[:, mbase + gg] = valid
            col128 += 1
    return idx, counts, mask, ng128


def _build_program(nblocks, jobs, ngathers, ng128, counts, reps=1, mode="full", debug=False):
    import concourse.mybir as mybir
    import concourse.tile as tile
    from concourse import bacc

    do_k = mode in ("full", "dma", "dmak")
    do_v = mode in ("full", "dma", "dmav")
    do_compute = mode in ("full", "compute")

    f32 = mybir.dt.float32
    bf16 = mybir.dt.bfloat16
    i16 = mybir.dt.int16
    Exp = mybir.ActivationFunctionType.Exp
    mult = mybir.AluOpType.mult

    nj = len(jobs)
    nc = bacc.Bacc("TRN2", target_bir_lowering=False)

    with tile.TileContext(nc) as tc:
        with tc.tile_pool(name="dram", bufs=1, space="DRAM") as dram:
            kcache_t = dram.tile([NUM_BLOCKS, KROW], bf16,
                                 kind="ExternalInput", name="kcache", uniquify=False)
            vcache_t = dram.tile([NUM_BLOCKS, VROW], bf16,
                                 kind="ExternalInput", name="vcache", uniquify=False)
            idx_t = dram.tile([128, nj * (BPG // 16)], i16,
                              kind="ExternalInput", name="idx", uniquify=False)
            mask_t = dram.tile([128, ng128 * BLOCK_SIZE * G], bf16,
                               kind="ExternalInput", name="mask", uniquify=False)
            qq_t = dram.tile([D, B * G], bf16, kind="ExternalInput", name="qq", uniquify=False)
            fold_t = dram.tile([16 * G, G], bf16, kind="ExternalInput", name="fold", uniquify=False)
            o_t = dram.tile([G, nj, D], f32, kind="ExternalOutput", name="o", uniquify=False)
            if debug:
                dbg_e = dram.tile([128, BLOCK_SIZE * G], f32, kind="ExternalOutput", name="dbg_e", uniquify=False)
                dbg_wt = dram.tile([128, BLOCK_SIZE * G], bf16, kind="ExternalOutput", name="dbg_wt", uniquify=False)
                dbg_den = dram.tile([BLOCK_SIZE * G, 1], bf16, kind="ExternalOutput", name="dbg_den", uniquify=False)
                dbg_osb = dram.tile([G, D], f32, kind="ExternalOutput", name="dbg_osb", uniquify=False)
                dbg_rec = dram.tile([G, 1], f32, kind="ExternalOutput", name="dbg_rec", uniquify=False)
                dbg_k = dram.tile([128, BLOCK_SIZE], bf16, kind="ExternalOutput", name="dbg_k", uniquify=False)
                dbg_v = dram.tile([128, D], bf16, kind="ExternalOutput", name="dbg_v", uniquify=False)

        with (
            tc.tile_pool(name="resident", bufs=1) as rpool,
            tc.tile_pool(name="kpool", bufs=3) as kpool,
            tc.tile_pool(name="vpool", bufs=3) as vpool,
            tc.tile_pool(name="wpool", bufs=8) as wpool,
            tc.tile_pool(name="small", bufs=2) as small_pool,
            tc.tile_pool(name="stps", bufs=2, space="PSUM") as stps_pool,
            tc.tile_pool(name="ops", bufs=2, space="PSUM") as ops_pool,
            tc.tile_pool(name="denps", bufs=2, space="PSUM") as denps_pool,
            tc.tile_pool(name="foldps", bufs=2, space="PSUM") as foldps_pool,
        ):
            idx_sb = rpool.tile([128, nj * (BPG // 16)], i16, tag="idx", name="idx_sb")
            mask_sb = rpool.tile([128, ng128 * BLOCK_SIZE * G], bf16, tag="mask", name="mask_sb")
            qq_sb = rpool.tile([D, B * G], bf16, tag="qq", name="qq_sb")
            fold_sb = rpool.tile([16 * G, G], bf16, tag="fold", name="fold_sb")
            ones_sb = rpool.tile([128, 1], bf16, tag="ones", name="ones_sb")
            oall_sb = rpool.tile([G, nj * D], f32, tag="oall", name="oall_sb")
            nc.sync.dma_start(idx_sb[:], idx_t[:])
            nc.sync.dma_start(mask_sb[:], mask_t[:])
            nc.sync.dma_start(qq_sb[:], qq_t[:])
            nc.sync.dma_start(fold_sb[:], fold_t[:])
            nc.vector.memset(ones_sb[:], 1.0)

            for _rep in range(reps):
                col128 = 0
                for jb, b in enumerate(jobs):
                    cnt, n16 = counts[jb]
                    ngroups = -(-n16 // 128)
                    o_ps = ops_pool.tile([G, D], f32, tag="o")
                    den_ps = denps_pool.tile([BLOCK_SIZE * G, 1], f32, tag="den")
                    nblk = ngroups * 128
                    ktile = kpool.tile([128, BLOCK_SIZE, nblk], bf16, tag="k")
                    vtile = vpool.tile([128, ngroups, VROW], bf16, tag="v")
                    if do_k:
                        nc.gpsimd.dma_gather(
                            ktile[:], kcache_t[:],
                            idx_sb[:, jb * 16 : jb * 16 + nblk // 16],
                            nblk, n16, KROW, transpose=True,
                        )
                    if do_v:
                        nc.gpsimd.dma_gather(
                            vtile[:], vcache_t[:],
                            idx_sb[:, jb * 16 : jb * 16 + nblk // 16],
                            nblk, n16, VROW,
                        )
                    if not do_compute:
                        col128 += ngroups
                        continue
                    for grp in range(ngroups):
                        nh = min(128, n16 - grp * 128)
                        first = grp == 0
                        last = grp == ngroups - 1
                        st16 = stps_pool.tile([128, BLOCK_SIZE * G], f32, tag="st")
                        for t in range(BLOCK_SIZE):
                            nc.tensor.matmul(
                                st16[:nh, t * G : (t + 1) * G],
                                lhsT=ktile[:, t, grp * 128 : grp * 128 + nh],
                                rhs=qq_sb[:, b * G : (b + 1) * G],
                                start=True, stop=True,
                            )
                        e16 = wpool.tile([128, BLOCK_SIZE * G], f32, tag="e")
                        nc.scalar.activation(e16[:nh], st16[:nh], Exp, scale=SCALE)
                        wt16 = wpool.tile([128, BLOCK_SIZE * G], bf16, tag="wt")
                        mbase = col128 * BLOCK_SIZE * G
                        nc.vector.tensor_tensor(
                            out=wt16[:nh], in0=e16[:nh],
                            in1=mask_sb[:nh, mbase : mbase + BLOCK_SIZE * G],
                            op=mult,
                        )
                        if debug and jb == 0 and grp == 0:
                            nc.sync.dma_start(dbg_e[:nh], e16[:nh])
                            nc.sync.dma_start(dbg_wt[:nh], wt16[:nh])
                            nc.sync.dma_start(dbg_k[:], ktile[:, :, 0])
                            nc.sync.dma_start(dbg_v[:], vtile[:, 0, 0:D])
                        for t in range(BLOCK_SIZE):
                            nc.tensor.matmul(
                                o_ps[:], lhsT=wt16[:nh, t * G : (t + 1) * G],
                                rhs=vtile[:nh, grp, t * D : (t + 1) * D],
                                start=first and t == 0,
                                stop=last and t == BLOCK_SIZE - 1,
                            )
                        nc.tensor.matmul(
                            den_ps[:], lhsT=wt16[:nh], rhs=ones_sb[:nh],
                            start=first, stop=last,
                        )
                        col128 += 1
                    # per-sequence epilogue: fold denominator, divide, store
                    o_sb = small_pool.tile([G, D], f32, tag="osb")
                    nc.vector.tensor_copy(o_sb[:], o_ps[:])
                    den_sb = small_pool.tile([BLOCK_SIZE * G, 1], bf16, tag="densb")
                    nc.vector.tensor_copy(den_sb[:], den_ps[:])
                    fold_ps = foldps_pool.tile([G, 1], f32, tag="fold")
                    nc.tensor.matmul(
                        fold_ps[:], lhsT=fold_sb[:], rhs=den_sb[:],
                        start=True, stop=True,
                    )
                    rec_sb = small_pool.tile([G, 1], f32, tag="rec")
                    nc.vector.reciprocal(rec_sb[:], fold_ps[:])
                    nc.vector.tensor_scalar(
                        oall_sb[:, jb * D : (jb + 1) * D], o_sb[:], rec_sb[:],
                        None, op0=mult
                    )
                    if debug and jb == 0:
                        nc.sync.dma_start(dbg_den[:], den_sb[:])
                        nc.sync.dma_start(dbg_osb[:], o_sb[:])
                        nc.sync.dma_start(dbg_rec[:], rec_sb[:])
                nc.sync.dma_start(o_t[:], oall_sb[:])

    nc.compile()
    return nc


def _host_prep(q, k, v, k_cache, v_cache, slot_mapping):
    """Returns per-core caches and q slices (all bf16)."""
    import ml_dtypes

    bf16 = ml_dtypes.bfloat16
    kc = k_cache.reshape(-1, HKV, D).copy()
    vc = v_cache.reshape(-1, HKV, D).copy()
    kc[slot_mapping] = k
    vc[slot_mapping] = v
    kc = kc.reshape(NUM_BLOCKS, BLOCK_SIZE, HKV, D)
    vc = vc.reshape(NUM_BLOCKS, BLOCK_SIZE, HKV, D)
    per_core = []
    for h in range(NCORES):
        kcache_h = np.ascontiguousarray(
            kc[:, :, h, :].reshape(NUM_BLOCKS, KROW)
        ).astype(bf16)
        vcache_h = np.ascontiguousarray(
            vc[:, :, h, :].reshape(NUM_BLOCKS, VROW)
        ).astype(bf16)
        qT_h = np.ascontiguousarray(
            q[:, h * G : (h + 1) * G, :].transpose(2, 0, 1).reshape(D, B * G)
        ).astype(bf16)
        per_core.append((kcache_h, vcache_h, qT_h))
    return per_core


def make_in_maps(q, k, v, k_cache, v_cache, slot_mapping, idx, mask):
    import ml_dtypes

    bf16 = ml_dtypes.bfloat16
    per_core = _host_prep(q, k, v, k_cache, v_cache, slot_mapping)
    mask_bf = mask.astype(bf16)
    fold = np.zeros((BLOCK_SIZE * G, G), dtype=np.float32)
    for u in range(BLOCK_SIZE):
        for g in range(G):
            fold[u * G + g, g] = 1.0
    fold = fold.astype(bf16)
    in_maps = []
    for h in range(NCORES):
        kcache_h, vcache_h, qq = per_core[h]
        in_maps.append(
            {
                "kcache": kcache_h,
                "vcache": vcache_h,
                "idx": idx,
                "mask": mask_bf,
                "qq": qq,
                "fold": fold,
            }
        )
    return in_maps


def assemble(results, jobs, context_lens):
    out = np.zeros((B, 1, H, D), dtype=np.float32)
    for h in range(NCORES):
        o_h = results[h]["o"]  # [G, nj, D]
        for jb, b in enumerate(jobs):
            if int(context_lens[b]) <= 0:
                continue
            out[b, 0, h * G : (h + 1) * G, :] = o_h[:, jb]
    return out


def kernel(q, k, v, k_cache, v_cache, slot_mapping, block_tables, context_lens):
    global LAST_EXEC_TIME_NS
    q = np.asarray(q, dtype=np.float32)
    k = np.asarray(k, dtype=np.float32)
    v = np.asarray(v, dtype=np.float32)
    k_cache = np.asarray(k_cache, dtype=np.float32)
    v_cache = np.asarray(v_cache, dtype=np.float32)
    slot_mapping = np.asarray(slot_mapping, dtype=np.int32)
    block_tables = np.asarray(block_tables, dtype=np.int32)
    context_lens = np.asarray(context_lens, dtype=np.int32)

    nblocks, jobs, ngathers = _plan(context_lens)
    if not jobs:
        return np.zeros((B, 1, H, D), dtype=np.float32)

    idx, counts, mask, ng128 = _host_tables(
        block_tables, context_lens, nblocks, jobs, ngathers
    )
    in_maps = make_in_maps(q, k, v, k_cache, v_cache, slot_mapping, idx, mask)
    nc = _build_program(nblocks, jobs, ngathers, ng128, counts)

    from concourse.bass_utils import run_bass_kernel_spmd

    res = run_bass_kernel_spmd(nc, in_maps, core_ids=list(range(NCORES)))
    LAST_EXEC_TIME_NS = res.exec_time_ns
    return assemble(res.results, jobs, context_lens)
